# revision 28
# baseline (speedup 1.0000x reference)
"""Trainium2 Bass kernel for nn_AttentionV2 (dense transformer attention block).

Reference computation (B=4, C=256, H=W=48, heads=8, d=32, N=2304):
  qk   = conv1x1(x, w_qk) -> q,k per head [d, N]
  v4   = conv1x1(x, w_v)
  pe   = dwconv3x3(v4, w_pe)            (depthwise, SAME)
  S    = q^T k * d^-0.5 ; P = softmax_j(S)
  out  = v @ P^T  (per head)
  y    = conv1x1(out + pe, w_proj)

Sharding: 8 cores = 4 batches x 2 spatial halves (rows 0-23 / 24-47).
Each core computes full k,v for its batch; per-core x ROLLED by y0 rows so
the SPMD program always works on "rows 0..23". Zero collectives.

v2 design (fp8 DoubleRow attention):
 - q/k convs run as fp8 DoubleRow matmuls (contraction 256 = 2 k-tiles of
   128 input channels, x8 = fp8 copy of x prepared host-side).
 - q/k stored fp8 in a d-split layout [16 partitions x 2 slots] per head, 4
   heads per 128-partition tile at 32-aligned bases; the ST matmul
   (S^T = k^T q) is one fp8 DoubleRow matmul per (head, j-chunk, i-chunk):
   0.5 cycles/row.
 - exp: j-chunk PAIRS [128, 2, icw] f32 PSUM -> E [128, 2, icw] fp8.
   ACT engine: true Exp with fp8 output.  DVE: Schraudolph exp2 trick
   (affine to uint8 e4m3 BITS, bitcast to fp8).  Split is load-balanced.
 - AV: fp8 DoubleRow with the j-pair as the two k-tiles (256 j per
   instruction); lhsT = [v^T | ones] so row 32 accumulates the softmax
   denominator.  Per-head PSUM accumulator [33, icw].
 - k conv bias dropped (softmax-invariant); b_v + b_pe folded into the
   proj bias host-side (linear).
 - Loop is isub(i-column)-major; per-isub tails: batched reciprocal over 8
   heads -> DMA partition-broadcast -> normalize.  pe (depthwise) runs on
   the Pool engine; pe-add + proj for isub i are deferred into isub 2's
   windows so the tap pipeline has time.
"""

import os
import numpy as np
from ml_dtypes import bfloat16, float8_e4m3

BASSDBG = os.environ.get("BASSDBG", "") == "1"

C = 256
HW = 48
N = HW * HW          # 2304
NH = N // 2          # 1152 per-core i-pixels
NHEADS = 8
D = 32
SCALE = D ** -0.5
NJ = N // 128        # 18 j-chunks
NJP = NJ // 2        # 9 j-pairs
ICW = 384
ISUBS = [0, 384, 768]
NCH384 = [k * 384 for k in range(6)]
LOG2E = 1.4426950408889634
QKS = 8.0            # host-side scale on wq/wk (fp8 subnormal avoidance)
EXPB = 55.57         # Schraudolph bias (56 - PWL mean correction)
WS = 32

_CACHE = {}


def _build_bass():
    import concourse.bass as bass
    import concourse.bacc as bacc
    import concourse.mybir as mybir
    from concourse import tile

    f32 = mybir.dt.float32
    bf16 = mybir.dt.bfloat16
    fp8 = mybir.dt.float8e4
    u8 = mybir.dt.uint8
    AF = mybir.ActivationFunctionType
    OP = mybir.AluOpType
    DR = mybir.MatmulPerfMode.DoubleRow

    nc = bacc.Bacc()

    x_d = nc.dram_tensor("x", [C, N], bf16, kind="ExternalInput")
    x8_d = nc.dram_tensor("x8", [128, 2 * N], fp8, kind="ExternalInput")
    w8_d = nc.dram_tensor("w8", [128, 8 * 256], fp8, kind="ExternalInput")
    wph_d = nc.dram_tensor("wph", [C, 512], bf16, kind="ExternalInput")
    wsb_d = nc.dram_tensor("wsb", [C, WS], f32, kind="ExternalInput")
    wdiag_d = nc.dram_tensor("wdiag", [128, 9 * 2 * 128], bf16, kind="ExternalInput")
    out_d = nc.dram_tensor("out", [C, NH], f32, kind="ExternalOutput")
    if BASSDBG:
        dbg_l = nc.dram_tensor("dbg_l", [NHEADS, NH], f32, kind="ExternalOutput")
        dbg_rl = nc.dram_tensor("dbg_rl", [NHEADS, NH], f32, kind="ExternalOutput")
        dbg_oU = nc.dram_tensor("dbg_oU", [C, NH], f32, kind="ExternalOutput")
        dbg_pin = nc.dram_tensor("dbg_pin", [C, NH], f32, kind="ExternalOutput")
        dbg_rlb = nc.dram_tensor("dbg_rlb", [C, NH], f32, kind="ExternalOutput")
        dbg_q8 = nc.dram_tensor("dbg_q8", [C, 2 * NH], f32, kind="ExternalOutput")
        dbg_k8 = nc.dram_tensor("dbg_k8", [C, 2 * N], f32, kind="ExternalOutput")
        dbg_y = nc.dram_tensor("dbg_y", [C, NH], f32, kind="ExternalOutput")
        dbg_pe = nc.dram_tensor("dbg_pe", [C, 384], f32, kind="ExternalOutput")
        dbg_ht = nc.dram_tensor("dbg_ht", [C, HW], f32, kind="ExternalOutput")

    with tile.TileContext(nc) as tc:
        with (
            tc.tile_pool(name="wts", bufs=1) as wp,
            tc.tile_pool(name="per", bufs=1) as pp,
        ):
            # ---- persistent weights
            w8 = wp.tile([128, 2, 2, 2, 2, 128], fp8, name="w8")
            nc.sync.dma_start(out=w8[:, :, :, :, :, :].rearrange("p a b c d e -> p (a b c d e)"),
                              in_=w8_d[:, :])
            wph = [wp.tile([128, 512], bf16, tag=f"wph{c}", name=f"wph{c}") for c in range(2)]
            wsb = [wp.tile([128, WS], f32, tag=f"wsb{c}", name=f"wsb{c}") for c in range(2)]
            for c in range(2):
                nc.sync.dma_start(out=wph[c][:, :], in_=wph_d[128 * c:128 * (c + 1), :])
                nc.sync.dma_start(out=wsb[c][:, :], in_=wsb_d[128 * c:128 * (c + 1), :])
            wvT = [wph[c][:, 0:256] for c in range(2)]
            wprojT = [wph[c][:, 256:512] for c in range(2)]
            wpe = [wsb[c][:, 0:9] for c in range(2)]
            bq8 = [[wsb[oc][:, 9 + s:10 + s] for s in range(2)] for oc in range(2)]
            bv = [wsb[c][:, 11:12] for c in range(2)]
            bvht = [wsb[c][:, 12:13] for c in range(2)]
            bproj = [wsb[c][:, 13:14] for c in range(2)]
            halo = [wsb[c][:, 14:16] for c in range(2)]
            bvhb = [wsb[c][:, 17:18] for c in range(2)]

            # ---- persistent activations
            x_sb = [pp.tile([128, N], bf16, tag=f"x{c}", name=f"x{c}") for c in range(2)]
            x8_sb = pp.tile([128, 2, N], fp8, tag="x8", name="x8")
            q8 = [pp.tile([128, 2, NH], fp8, tag=f"q8{c}", name=f"q8{c}") for c in range(2)]
            k8 = [pp.tile([128, 2, N], fp8, tag=f"k8{c}", name=f"k8{c}") for c in range(2)]
            vT8 = pp.tile([128, NJP, NHEADS, 2, 64], fp8, tag="vT8", name="vT8")
            # v4p: zero-padded dwconv input.  row 0 = top halo, rows 1..24 =
            # v4 (+bias), row 25 = bottom halo; cols 0 and 49 stay zero.
            v4p = [pp.tile([128, 26, 50], bf16, tag=f"v4{c}", name=f"v4{c}") for c in range(2)]
            wdiag = pp.tile([128, 9, 2, 128], bf16, tag="wdiag", name="wdiag")
            outU = [pp.tile([128, NH], f32, tag=f"oU{c}", name=f"oU{c}") for c in range(2)]
            # l/rl: head 4oc+g lives at partition 32g of tile oc (32-aligned
            # so ACT can write the row and the PE broadcast can read it)
            l2 = [pp.tile([128, NH], f32, tag=f"l2{c}", name=f"l2{c}") for c in range(2)]
            rl2 = [pp.tile([128, NH], f32, tag=f"rl2{c}", name=f"rl2{c}") for c in range(2)]
            scr2 = pp.tile([128, NH], f32, tag="scr2", name="scr2")
            ones128 = pp.tile([128, 32], f32, tag="ones", name="ones")
            proj_in = [pp.tile([128, NH], bf16, tag=f"pin{c}", name=f"pin{c}") for c in range(2)]
            y_sb = [pp.tile([128, NH], f32, tag=f"y{c}", name=f"y{c}") for c in range(2)]

            nc.vector.memset(vT8[:, :, :, :, 32:33], 1.0)
            nc.vector.memset(ones128[:, :], 1.0)
            for c in range(2):
                nc.vector.memset(v4p[c][:, :, :].rearrange("p a b -> p (a b)"), 0.0)
            for c in range(2):
                nc.vector.memset(l2[c][:, :], 1.0)
            for c0 in range(0, 2304, 768):
                nc.sync.dma_start(out=wdiag[:, :, :, :].rearrange("p a b c -> p (a b c)")[:, c0:c0 + 768],
                                  in_=wdiag_d[:, c0:c0 + 768])

            for s0 in range(0, N, 768):
                nc.sync.dma_start(out=x8_sb[:, :, :].rearrange("p a b -> p (a b)")[:, 2 * s0:2 * s0 + 1536],
                                  in_=x8_d[:, 2 * s0:2 * s0 + 1536])
            for s0 in range(0, N, 768):
                for c in range(2):
                    nc.sync.dma_start(out=x_sb[c][:, s0:s0 + 768],
                                      in_=x_d[128 * c:128 * (c + 1), s0:s0 + 768])

            # ================= emitters =================
            def qk8_chunk(kind, oc, so, c0, w, pool):
                pt = pool.tile([128, 384], f32, tag="pps", name="pps")
                nc.tensor.matmul(
                    pt[:, :w],
                    w8[:, kind, oc, so, :, :],
                    x8_sb[:, :, c0:c0 + w],
                    start=True, stop=True, perf_mode=DR,
                )
                if kind == 0:
                    nc.vector.tensor_scalar(q8[oc][:, so, c0:c0 + w], pt[:, :w],
                                            bq8[oc][so][:, :], None, OP.add)
                else:
                    nc.vector.tensor_copy(k8[oc][:, so, c0:c0 + w], pt[:, :w])

            def vt8_chunk(j, pool):
                pt = pool.tile([128, 384], f32, tag="pps", name="pps")
                for c in range(2):
                    nc.tensor.matmul(
                        pt[:, :256],
                        x_sb[c][:, 128 * j:128 * (j + 1)],
                        wvT[c][:, :],
                        start=(c == 0), stop=(c == 1),
                    )
                nc.vector.tensor_copy(
                    vT8[:, j // 2, :, j % 2, 0:32],
                    pt[:, :256].rearrange("p (h d) -> p h d", d=32),
                )

            def v4_chunk(oc, c0, w, pool):
                # w=384 chunks land on v4p rows 1+c0/48 .. +8; the 48-wide
                # chunk at c0=1152 is image row 24 -> bottom halo row 25.
                pt = pool.tile([128, 384], f32, tag="pps", name="pps")
                for c in range(2):
                    nc.tensor.matmul(
                        pt[:, :w],
                        wvT[c][:, 128 * oc:128 * (oc + 1)],
                        x_sb[c][:, c0:c0 + w],
                        start=(c == 0), stop=(c == 1),
                    )
                r0 = 1 + c0 // 48
                if w == 384:
                    nc.scalar.activation(v4p[oc][:, r0:r0 + 8, 1:49],
                                         pt[:, :w].rearrange("p (a b) -> p a b", b=48),
                                         AF.Identity, bias=bv[oc][:, :])
                else:
                    nc.scalar.activation(v4p[oc][:, 25:26, 1:49], pt[:, :48],
                                         AF.Identity, bias=bvhb[oc][:, :],
                                         scale=halo[oc][:, 1:2])

            def v4_halo(oc, pool):
                # top halo: conv of (rolled) image row 47
                pt = pool.tile([128, 384], f32, tag="pps", name="pps")
                for c in range(2):
                    nc.tensor.matmul(
                        pt[:, :48],
                        wvT[c][:, 128 * oc:128 * (oc + 1)],
                        x_sb[c][:, 47 * 48:48 * 48],
                        start=(c == 0), stop=(c == 1),
                    )
                nc.scalar.activation(v4p[oc][:, 0:1, 1:49], pt[:, :48], AF.Identity,
                                     bias=bvht[oc][:, :], scale=halo[oc][:, 0:1])

            ALL9 = [(0, 0)] + [(dy, dx) for dy in (-1, 0, 1) for dx in (-1, 0, 1)
                               if not (dy == 0 and dx == 0)]

            def emit_pe_taps(isub, oc, pool):
                # depthwise 3x3 conv on the PE: 9 diagonal-matrix matmuls
                # accumulating into one PSUM bank (out rows 8i..8i+8), inputs
                # from the zero-padded v4p.
                r0 = 8 * isub
                pe_ps = pool.tile([128, 8, 48], f32, tag="pps", name="pps")
                for n, (dy, dx) in enumerate(ALL9):
                    wi = 3 * (dy + 1) + (dx + 1)
                    nc.tensor.matmul(
                        pe_ps[:, :, :].rearrange("p a b -> p (a b)"),
                        wdiag[:, wi, oc, :],
                        v4p[oc][:, r0 + dy + 1:r0 + dy + 9, 1 + dx:49 + dx],
                        start=(n == 0), stop=(n == len(ALL9) - 1),
                    )
                return pe_ps

            # ================= stage 1: minimal prefix =================
            with tc.tile_pool(name="ps1", bufs=2, space="PSUM") as ps1:
                for so in range(2):
                    qk8_chunk(1, 0, so, 0, 384, ps1)       # k8[0] chunk 0
                for so in range(2):
                    for oc in range(2):
                        qk8_chunk(0, oc, so, 0, 384, ps1)  # q8 isub 0
                for j in range(4):
                    vt8_chunk(j, ps1)
                for so in range(2):
                    qk8_chunk(1, 0, so, 384, 384, ps1)     # k8[0] chunk 1

            # ================= fillers =================
            def mk(fn, *args):
                return lambda pool: fn(*args, pool)

            FILL = {}
            # Emission-order discipline: a tile WRITE must be emitted before
            # any consumer READ (the Tile framework only syncs such pairs).
            # grp0/isub0 consumes k8[0] j-chunks (ST u=2t reads j=2t,2t+1;
            # conv chunk ci covers j=3ci..3ci+3) and vT8 pairs (AV at
            # iteration 2t+2).  Interleave so everything lands in time.
            FILL[(0, 0)] = (
                [mk(qk8_chunk, 1, 0, so, NCH384[2], 384) for so in range(2)]
                + [mk(vt8_chunk, 4), mk(vt8_chunk, 5)]
                + [mk(qk8_chunk, 1, 0, so, NCH384[3], 384) for so in range(2)]
                + [mk(vt8_chunk, 6), mk(vt8_chunk, 7)]
                + [mk(vt8_chunk, 8), mk(vt8_chunk, 9)]
                + [mk(qk8_chunk, 1, 0, so, NCH384[4], 384) for so in range(2)]
                + [mk(vt8_chunk, 10), mk(vt8_chunk, 11)]
                + [mk(qk8_chunk, 1, 0, so, NCH384[5], 384) for so in range(2)]
                + [mk(vt8_chunk, j) for j in range(12, 18)]
            )
            FILL[(0, 1)] = ([mk(qk8_chunk, 1, 1, so, NCH384[ci], 384)
                             for ci in range(6) for so in range(2)]
                            + [mk(v4_chunk, 0, 0, 384)])
            FILL[(0, 2)] = ([mk(qk8_chunk, 0, oc, so, 384, 384)
                             for oc in range(2) for so in range(2)]
                            + [mk(v4_chunk, 0, 384, 384), mk(v4_chunk, 0, 768, 384),
                               mk(v4_chunk, 0, 1152, 96), mk(v4_halo, 0)])
            FILL[(0, 3)] = [mk(v4_chunk, 1, 0, 384), mk(v4_chunk, 1, 384, 384),
                            mk(v4_chunk, 1, 768, 384), mk(v4_chunk, 1, 1152, 96),
                            mk(v4_halo, 1)]
            FILL[(1, 0)] = []
            FILL[(1, 1)] = []
            FILL[(1, 2)] = [mk(qk8_chunk, 0, oc, so, 768, 384)
                            for oc in range(2) for so in range(2)]
            FILL[(1, 3)] = []
            for g in range(4):
                FILL[(2, g)] = []

            EXPAT = {}
            for isub in range(3):
                for grp in range(4):
                    if isub == 0 and grp < 2:
                        EXPAT[(isub, grp)] = "AAAADAAADAAAADAAAD"
                    else:
                        EXPAT[(isub, grp)] = "ADAADAADADAADAADAD"

            # ================= stage 2: attention =================
            with (
                tc.tile_pool(name="ep", bufs=3) as ep,
                tc.tile_pool(name="stp", bufs=2, space="PSUM") as stp,
                tc.tile_pool(name="ava", bufs=1, space="PSUM") as ava,
                tc.tile_pool(name="ps3", bufs=2, space="PSUM") as ps3,
            ):
                def emit_norm_all(isub):
                    # recip over both l2 tiles, PE-matmul broadcast into PSUM,
                    # then normalize.  No DMA round-trip.
                    i0 = ISUBS[isub]
                    for oc2 in range(2):
                        nc.vector.reciprocal_approx_accurate(rl2[oc2][:, i0:i0 + ICW],
                                                             l2[oc2][:, i0:i0 + ICW],
                                                             scr2[:, i0:i0 + ICW])
                    for oc2 in range(2):
                        rlbps = ps3.tile([128, 384], f32, tag="pps", name="pps")
                        for g in range(4):
                            nc.tensor.matmul(
                                rlbps[32 * g:32 * g + 32, :ICW],
                                ones128[32 * g:32 * g + 1, :],
                                rl2[oc2][32 * g:32 * g + 1, i0:i0 + ICW],
                                start=True, stop=True,
                                tile_position=(32 * g, 32 * g),
                            )
                        nc.vector.tensor_tensor(
                            proj_in[oc2][:, i0:i0 + ICW], outU[oc2][:, i0:i0 + ICW],
                            rlbps[:, :ICW], OP.mult,
                        )

                def emit_proj(isub):
                    i0 = ISUBS[isub]
                    for oc2 in range(2):
                        pe_ps = emit_pe_taps(isub, oc2, ps3)
                        pef = pe_ps[:, :, :].rearrange("p a b -> p (a b)")
                        nc.vector.tensor_tensor(
                            proj_in[oc2][:, i0:i0 + ICW], proj_in[oc2][:, i0:i0 + ICW],
                            pef[:, :ICW], OP.add,
                        )
                    for oc2 in range(2):
                        pt = ps3.tile([128, 384], f32, tag="pps", name="pps")
                        for c in range(2):
                            nc.tensor.matmul(
                                pt[:, :ICW],
                                wprojT[c][:, 128 * oc2:128 * (oc2 + 1)],
                                proj_in[c][:, i0:i0 + ICW],
                                start=(c == 0), stop=(c == 1),
                            )
                        nc.scalar.activation(y_sb[oc2][:, i0:i0 + ICW], pt[:, :ICW],
                                             AF.Identity, bias=bproj[oc2][:, :])
                        nc.sync.dma_start(out=out_d[128 * oc2:128 * (oc2 + 1), i0:i0 + ICW],
                                          in_=y_sb[oc2][:, i0:i0 + ICW])

                for isub in range(3):
                    i0 = ISUBS[isub]
                    for grp in range(4):
                        oc = grp // 2
                        heads = [2 * grp, 2 * grp + 1]
                        pat = EXPAT[(isub, grp)]
                        fillers = FILL.get((isub, grp), [])
                        fi = [0]

                        def filler():
                            if fi[0] < len(fillers):
                                fillers[fi[0]](ps3)
                                fi[0] += 1

                        avl = {h: ava.tile([33, 512], f32, tag=f"av{h % 2}",
                                           name=f"av{h % 2}") for h in heads}
                        units = [(h, t) for t in range(NJP) for h in heads]
                        sts, ets = {}, {}

                        def emit_st(u):
                            h, t = units[u]
                            g = h % 4
                            st = stp.tile([128, 2, 512], f32, tag="st", name="st")
                            for e in range(2):
                                j = 2 * t + e
                                nc.tensor.matmul(
                                    st[:, e, :ICW],
                                    k8[oc][32 * g:32 * g + 16, :, 128 * j:128 * (j + 1)],
                                    q8[oc][32 * g:32 * g + 16, :, i0:i0 + ICW],
                                    start=True, stop=True, perf_mode=DR,
                                    tile_position=(32 * g, 0),
                                )
                            sts[u] = st

                        def emit_exp(u):
                            h, t = units[u]
                            st = sts.pop(u)
                            et = ep.tile([128, 2, 512], fp8, tag="E", name="E")
                            if pat[u] == 'A':
                                nc.scalar.activation(et[:, :, :ICW], st[:, :, :ICW],
                                                     AF.Exp, scale=SCALE / (QKS * QKS))
                            else:
                                etu = et[:, :, :].bitcast(u8)
                                nc.vector.tensor_scalar(
                                    etu[:, :, :ICW], st[:, :, :ICW],
                                    8.0 * LOG2E * SCALE / (QKS * QKS),
                                    EXPB, OP.mult, OP.add)
                            ets[u] = et

                        def emit_av(u):
                            h, t = units[u]
                            et = ets.pop(u)
                            nc.tensor.matmul(
                                avl[h][:, :ICW],
                                vT8[:, t, h, :, 0:33],
                                et[:, :, :ICW],
                                start=(t == 0), stop=(t == NJP - 1),
                                perf_mode=DR,
                            )

                        for u in range(len(units)):
                            emit_st(u)
                            filler()
                            if u >= 1:
                                emit_exp(u - 1)
                            if u >= 2:
                                emit_av(u - 2)
                            filler()
                        emit_exp(len(units) - 1)
                        while fi[0] < len(fillers):
                            fillers[fi[0]](ps3)
                            fi[0] += 1
                        emit_av(len(units) - 2)
                        emit_av(len(units) - 1)

                        # drain: copy outU (DVE) + l rows (ACT -> DMA) off PSUM
                        for h in heads:
                            rr = 32 * (h % 4)
                            nc.vector.tensor_copy(outU[oc][rr:rr + 32, i0:i0 + ICW],
                                                  avl[h][0:32, :ICW])
                            nc.scalar.copy(l2[oc][32 * (h % 4):32 * (h % 4) + 1,
                                                  i0:i0 + ICW],
                                           avl[h][32:33, :ICW])

                        # staggered tail work
                        if isub == 2:
                            if grp == 0:
                                emit_proj(0)
                            elif grp == 2:
                                emit_proj(1)

                    emit_norm_all(isub)
                # final tail
                emit_proj(2)

                if BASSDBG:
                    nc.sync.dma_start(out=dbg_l[:, :], in_=l_sb[:, :])
                    nc.sync.dma_start(out=dbg_rl[:, :], in_=rl_sb[:, :])
                    dtmp = [pp.tile([128, NH], f32, tag=f"dt{c}", name=f"dt{c}")
                            for c in range(2)]
                    dtq = [pp.tile([128, 2 * NH], f32, tag=f"dtq{c}", name=f"dtq{c}")
                           for c in range(2)]
                    dtk = [pp.tile([128, 2 * N], f32, tag=f"dtk{c}", name=f"dtk{c}")
                           for c in range(2)]
                    for c in range(2):
                        nc.vector.tensor_copy(dtmp[c][:, :], outU[c][:, :])
                        nc.sync.dma_start(out=dbg_oU[128 * c:128 * (c + 1), :], in_=dtmp[c][:, :])
                    for c in range(2):
                        nc.vector.tensor_copy(dtq[c][:, :],
                                              q8[c][:, :, :].rearrange("p a b -> p (a b)"))
                        nc.sync.dma_start(out=dbg_q8[128 * c:128 * (c + 1), :], in_=dtq[c][:, :])
                        nc.vector.tensor_copy(dtk[c][:, :],
                                              k8[c][:, :, :].rearrange("p a b -> p (a b)"))
                        nc.sync.dma_start(out=dbg_k8[128 * c:128 * (c + 1), :], in_=dtk[c][:, :])
                    for c in range(2):
                        nc.vector.tensor_copy(dtmp[c][:, :], proj_in[c][:, :])
                        nc.sync.dma_start(out=dbg_pin[128 * c:128 * (c + 1), :], in_=dtmp[c][:, :])
                        nc.sync.dma_start(out=dbg_rlb[128 * c:128 * (c + 1), :], in_=rlb[c][:, :])
                    for c in range(2):
                        nc.sync.dma_start(out=dbg_y[128 * c:128 * (c + 1), :], in_=y_sb[c][:, :])

    nc.finalize()
    return nc


def _prep_inputs(x, w_qk, b_qk, w_v, b_v, w_pe, b_pe, w_proj, b_proj):
    f = np.float32
    wqk2 = w_qk[:, :, 0, 0].reshape(NHEADS, 2 * D, C)
    bqk2 = b_qk.reshape(NHEADS, 2 * D)
    wq = wqk2[:, :D, :]            # [h, d, inch]
    wk = wqk2[:, D:, :]
    bq = bqk2[:, :D]               # [h, d]

    # fp8 DR conv weights [128si_part, kind, oc, so, si, col]
    w8 = np.zeros((128, 2, 2, 2, 2, 128), dtype=f)
    for kind, wsrc in ((0, wq), (1, wk)):
        for oc in range(2):
            for so in range(2):
                for g in range(4):
                    h = 4 * oc + g
                    for p in range(16):
                        d = 16 * so + p
                        w8[:, kind, oc, so, 0, 32 * g + p] = wsrc[h, d, 0:128] * QKS
                        w8[:, kind, oc, so, 1, 32 * g + p] = wsrc[h, d, 128:256] * QKS
    w8 = w8.astype(float8_e4m3)

    wph = np.zeros((C, 512), dtype=f)
    wph[:, 0:256] = w_v[:, :, 0, 0].T
    wph[:, 256:512] = w_proj[:, :, 0, 0].T
    wph = wph.astype(bfloat16)

    bproj_f = b_proj + w_proj[:, :, 0, 0] @ (b_v + b_pe)

    # depthwise conv as diag matmuls: wdiag[p, tap, oc, col] = (p==col)*wpe[128oc+p, tap]
    wpe9 = w_pe[:, 0].reshape(C, 9)
    wdiag = np.zeros((128, 9, 2, 128), dtype=f)
    for oc in range(2):
        for t in range(9):
            np.fill_diagonal(wdiag[:, t, oc, :], wpe9[128 * oc:128 * (oc + 1), t])
    wdiag = wdiag.astype(bfloat16)

    base = np.zeros((C, WS), dtype=f)
    base[:, 0:9] = w_pe[:, 0].reshape(C, 9)
    for oc in range(2):
        for so in range(2):
            for g in range(4):
                for p in range(16):
                    base[128 * oc + 32 * g + p, 9 + so] = bq[4 * oc + g, 16 * so + p] * QKS
    base[:, 11] = b_v
    base[:, 13] = bproj_f

    in_maps = []
    for core in range(8):
        b, half = core // 2, core % 2
        y0 = 24 * half
        xb = x[b].reshape(C, HW, HW).astype(f)
        xr = np.concatenate([xb[:, y0:, :], xb[:, :y0, :]], axis=1).reshape(C, N)
        halo_top = 1.0 if half == 1 else 0.0
        halo_bot = 1.0 if half == 0 else 0.0
        wsb = base.copy()
        wsb[:, 12] = halo_top * b_v
        wsb[:, 14] = halo_top
        wsb[:, 15] = halo_bot
        wsb[:, 17] = halo_bot * b_v
        x8 = np.stack([xr[0:128], xr[128:256]], axis=1)  # [128, 2, N]
        in_maps.append({
            "x": np.ascontiguousarray(xr.astype(bfloat16)),
            "x8": np.ascontiguousarray(x8.astype(float8_e4m3).reshape(128, 2 * N)),
            "w8": np.ascontiguousarray(w8.reshape(128, 8 * 256)),
            "wph": wph, "wsb": wsb,
            "wdiag": np.ascontiguousarray(wdiag.reshape(128, 9 * 2 * 128)),
        })
    return in_maps


def kernel(**inputs):
    from concourse.bass_utils import run_bass_kernel_spmd

    if "nc" not in _CACHE:
        _CACHE["nc"] = _build_bass()
    nc = _CACHE["nc"]

    in_maps = _prep_inputs(**inputs)
    res = run_bass_kernel_spmd(nc, in_maps, core_ids=list(range(8)))
    y = np.empty((4, C, HW, HW), dtype=np.float32)
    for core in range(8):
        b, half = core // 2, core % 2
        y0 = 24 * half
        y[b][:, y0:y0 + 24, :] = res.results[core]["out"].reshape(C, 24, HW)
    return y


# revision 29
# speedup vs baseline: 1.1934x; 1.1934x over previous
"""Trainium2 Bass kernel for nn_AttentionV2 (dense transformer attention block).

Reference computation (B=4, C=256, H=W=48, heads=8, d=32, N=2304):
  qk   = conv1x1(x, w_qk) -> q,k per head [d, N]
  v4   = conv1x1(x, w_v)
  pe   = dwconv3x3(v4, w_pe)            (depthwise, SAME)
  S    = q^T k * d^-0.5 ; P = softmax_j(S)
  out  = v @ P^T  (per head)
  y    = conv1x1(out + pe, w_proj)

Sharding: 8 cores = 4 batches x 2 spatial halves (rows 0-23 / 24-47).
Each core computes full k,v for its batch (attention rows are independent
given full k/v) -> zero collectives. The per-core x is ROLLED by y0 rows so
the SPMD program always works on "rows 0..23"; softmax/attention are
permutation-invariant in j. Halo rows for the depthwise conv are gated by
per-core 0/1 flag inputs.

Attention is computed in "ST" layout (S^T = k^T q, j on partitions):
 - PE matmul k[:,jc]^T q -> ST psum [128 j, 1152 i]
 - ACT exp(SCALE*ST) psum -> SBUF E (bf16), no max-subtraction (|S*scale|<~5)
 - PE AV matmul with lhsT = [v^T | ones] accumulated over j-chunks gives both
   the unnormalized output (rows 0..32) and the softmax denominator (row 32).
"""

import os
import numpy as np
from ml_dtypes import bfloat16

LOG2E = 1.4426950408889634
EXPB16 = 16252.3   # 127*128 - PWL mean correction, bf16 exp2 bit trick

C = 256
HW = 48
N = HW * HW          # 2304
NH = N // 2          # 1152 per-core i-pixels
NHEADS = 8
D = 32
SCALE = D ** -0.5
NJ = N // 128        # 18 j-chunks
ISUBS = [(0, 384), (384, 384), (768, 384)]
NCH384 = [(k * 384, 384) for k in range(6)]   # full-image conv col chunks
WPACK = 1041

_CACHE = {}


def _build_bass():
    import concourse.bass as bass
    import concourse.bacc as bacc
    import concourse.mybir as mybir
    from concourse import tile

    f32 = mybir.dt.float32
    bf16 = mybir.dt.bfloat16
    AF = mybir.ActivationFunctionType
    OP = mybir.AluOpType

    nc = bacc.Bacc()

    x_d = nc.dram_tensor("x", [C, N], bf16, kind="ExternalInput")
    # all weights/biases packed into one tensor: [wqkT|wvT|wprojT|wpe|biases|halo]
    wpack_d = nc.dram_tensor("wpack", [C, WPACK], f32, kind="ExternalInput")
    # bf16 copy of the matmul weights [wqkT|wvT|wprojT]
    wpackh_d = nc.dram_tensor("wpackh", [C, 1024], bf16, kind="ExternalInput")
    kzero_d = nc.dram_tensor("kzero", [C, N], bf16, kind="ExternalInput")
    out_d = nc.dram_tensor("out", [C, NH], f32, kind="ExternalOutput")

    with tile.TileContext(nc) as tc:
        with (
            tc.tile_pool(name="wts", bufs=1) as wp,
            tc.tile_pool(name="per", bufs=1) as pp,
        ):
            # ---- persistent weight/bias tiles (single packed DMA per chunk)
            wsb = [wp.tile([128, WPACK], f32, tag=f"wsb{c}", name=f"wsb{c}") for c in range(2)]
            wph = [wp.tile([128, 1024], bf16, tag=f"wph{c}", name=f"wph{c}") for c in range(2)]
            for c in range(2):
                nc.sync.dma_start(out=wsb[c][:, :], in_=wpack_d[128 * c:128 * (c + 1), :])
                nc.sync.dma_start(out=wph[c][:, :], in_=wpackh_d[128 * c:128 * (c + 1), :])
            wqkT = [wph[c][:, 0:512] for c in range(2)]
            wvT = [wph[c][:, 512:768] for c in range(2)]
            wprojT = [wph[c][:, 768:1024] for c in range(2)]
            wpe = [wsb[c][:, 1024:1033] for c in range(2)]
            bq = [wsb[c][:, 1033:1034] for c in range(2)]
            bk = [wsb[c][:, 1034:1035] for c in range(2)]
            bv = [wsb[c][:, 1035:1036] for c in range(2)]
            bvpe = [wsb[c][:, 1036:1037] for c in range(2)]
            bproj = [wsb[c][:, 1037:1038] for c in range(2)]
            bvht = [wsb[c][:, 1038:1039] for c in range(2)]
            halo = [wsb[c][:, 1039:1041] for c in range(2)]

            # ---- persistent activations
            q_sb = [pp.tile([128, NH], bf16, tag=f"q{c}", name=f"q{c}") for c in range(2)]
            k_sb = [pp.tile([128, N], bf16, tag=f"k{c}", name=f"k{c}") for c in range(2)]
            vT = pp.tile([128, NJ, NHEADS, 33], bf16, tag="vT", name="vT")
            v4 = [pp.tile([128, 26, HW], f32, tag=f"v4{c}", name=f"v4{c}") for c in range(2)]
            htop = [pp.tile([128, 1, HW], f32, tag=f"htop{c}", name=f"htop{c}") for c in range(2)]
            hbot = [pp.tile([128, 1, HW], f32, tag=f"hbot{c}", name=f"hbot{c}") for c in range(2)]
            pe = [pp.tile([128, 24, HW], f32, tag=f"pe{c}", name=f"pe{c}") for c in range(2)]
            outU = [pp.tile([128, NH], f32, tag=f"outU{c}", name=f"outU{c}") for c in range(2)]
            l_g = [pp.tile([4, NH], f32, tag=f"l{g}", name=f"l{g}") for g in range(2)]
            rl_g = [pp.tile([4, NH], f32, tag=f"rl{g}", name=f"rl{g}") for g in range(2)]
            rscr = pp.tile([4, NH], f32, tag="rscr", name="rscr")
            rl8 = [pp.tile([1, NH], f32, tag=f"rl8_{h}", name=f"rl8_{h}") for h in range(NHEADS)]
            rlb_sb = [pp.tile([128, NH], f32, tag=f"rlb{g}", name=f"rlb{g}") for g in range(2)]
            proj_in = [pp.tile([128, NH], bf16, tag=f"pin{c}", name=f"pin{c}") for c in range(2)]

            nc.vector.memset(vT[:, :, :, 32:33], 1.0)
            k_pad = [pp.tile([128, N], bf16, tag=f"kp{h}", name=f"kp{h}")
                     for h in range(NHEADS)]
            for h in range(2):
                nc.sync.dma_start(out=k_pad[h][:, :], in_=kzero_d[0:128, :])

            # ================= stage 1: minimal prefix =================
            # Only what the first ST/exp needs runs before attention: x, the
            # k/q convs for heads 0/1 and the first two vT chunks. Everything
            # else becomes "filler" closures interleaved into the attention
            # loop so the PE never idles and the first exp starts early.
            xpool = tc.tile_pool(name="xp", bufs=1)
            xp = xpool.__enter__()
            x_sb = [xp.tile([128, N], bf16, tag=f"x{c}", name=f"x{c}") for c in range(2)]
            for s in range(0, N, 768):
                for c in range(2):
                    nc.sync.dma_start(out=x_sb[c][:, s:s + 768],
                                      in_=x_d[128 * c:128 * (c + 1), s:s + 768])

            def qk_chunk(t, c0, w, pool):
                is_q = t < 2
                oc = t % 2
                dst = q_sb[oc] if is_q else k_sb[oc]
                bias = bq[oc] if is_q else bk[oc]
                pt = pool.tile([128, 384], f32, tag="pps", name="pps")
                for c in range(2):
                    nc.tensor.matmul(
                        pt[:, :w],
                        wqkT[c][:, 128 * t:128 * (t + 1)],
                        x_sb[c][:, c0:c0 + w],
                        start=(c == 0), stop=(c == 1),
                    )
                nc.vector.tensor_scalar(dst[:, c0:c0 + w], pt[:, :w], bias[:, :], None, OP.add)

            def vt_chunk(j, pool):
                pt = pool.tile([128, 384], f32, tag="pps", name="pps")
                for c in range(2):
                    nc.tensor.matmul(
                        pt[:, :256],
                        x_sb[c][:, 128 * j:128 * (j + 1)],
                        wvT[c][:, :],
                        start=(c == 0), stop=(c == 1),
                    )
                nc.vector.tensor_copy(
                    vT[:, j, :, 0:32],
                    pt[:, :256].rearrange("p (h d) -> p h d", d=32),
                )

            def v4_chunk(oc, c0, w, pool):
                pt = pool.tile([128, 384], f32, tag="pps", name="pps")
                for c in range(2):
                    nc.tensor.matmul(
                        pt[:, :w],
                        wvT[c][:, 128 * oc:128 * (oc + 1)],
                        x_sb[c][:, c0:c0 + w],
                        start=(c == 0), stop=(c == 1),
                    )
                v4f = v4[oc][:, :, :].rearrange("p a b -> p (a b)")
                nc.vector.tensor_scalar(v4f[:, c0:c0 + w], pt[:, :w], bv[oc][:, :], None, OP.add)

            def v4_halo(oc, pool):
                pt = pool.tile([128, 384], f32, tag="pps", name="pps")
                for c in range(2):
                    nc.tensor.matmul(
                        pt[:, :48],
                        wvT[c][:, 128 * oc:128 * (oc + 1)],
                        x_sb[c][:, 47 * 48:48 * 48],
                        start=(c == 0), stop=(c == 1),
                    )
                nc.scalar.activation(htop[oc][:, 0, :], pt[:, :48], AF.Identity,
                                     bias=bvht[oc][:, :], scale=halo[oc][:, 0:1])
                nc.scalar.activation(hbot[oc][:, 0, :], v4[oc][:, 24, :], AF.Copy,
                                     scale=halo[oc][:, 1:2])

            def pe_taps(oc):
                w9 = wpe[oc]
                acc = pe[oc]
                src = v4[oc]
                nc.vector.tensor_scalar(acc[:, :, :], src[:, 0:24, :], w9[:, 4:5], None, OP.mult)
                taps = [
                    (-1, -1, (1, 24), (0, 23), (1, 48), (0, 47)),
                    (-1, 0, (1, 24), (0, 23), (0, 48), (0, 48)),
                    (-1, 1, (1, 24), (0, 23), (0, 47), (1, 48)),
                    (0, -1, (0, 24), (0, 24), (1, 48), (0, 47)),
                    (0, 1, (0, 24), (0, 24), (0, 47), (1, 48)),
                    (1, -1, (0, 23), (1, 24), (1, 48), (0, 47)),
                    (1, 0, (0, 23), (1, 24), (0, 48), (0, 48)),
                    (1, 1, (0, 23), (1, 24), (0, 47), (1, 48)),
                ]
                for (dy, dx, oy, iy, ox, ix) in taps:
                    wap = w9[:, 3 * (dy + 1) + (dx + 1):3 * (dy + 1) + (dx + 1) + 1]
                    nc.vector.scalar_tensor_tensor(
                        acc[:, oy[0]:oy[1], ox[0]:ox[1]],
                        src[:, iy[0]:iy[1], ix[0]:ix[1]],
                        wap,
                        acc[:, oy[0]:oy[1], ox[0]:ox[1]],
                        OP.mult, OP.add,
                    )
                for (dx, ox, ix) in [(-1, (1, 48), (0, 47)), (0, (0, 48), (0, 48)), (1, (0, 47), (1, 48))]:
                    wap = w9[:, (dx + 1):(dx + 2)]
                    nc.vector.scalar_tensor_tensor(
                        acc[:, 0:1, ox[0]:ox[1]], htop[oc][:, :, ix[0]:ix[1]],
                        wap, acc[:, 0:1, ox[0]:ox[1]], OP.mult, OP.add,
                    )
                    wap = w9[:, 6 + (dx + 1):6 + (dx + 2)]
                    nc.vector.scalar_tensor_tensor(
                        acc[:, 23:24, ox[0]:ox[1]], hbot[oc][:, :, ix[0]:ix[1]],
                        wap, acc[:, 23:24, ox[0]:ox[1]], OP.mult, OP.add,
                    )

            def kpad_dma(heads):
                for h in heads:
                    if h >= 2:
                        nc.sync.dma_start(out=k_pad[h][:, :], in_=kzero_d[0:128, :])
                for h in heads:
                    r = 32 * (h % 4)
                    nc.sync.dma_start(out=k_pad[h][r:r + 32, :],
                                      in_=k_sb[h // 4][r:r + 32, :])

            with tc.tile_pool(name="ps1", bufs=2, space="PSUM") as ps1:
                for (c0, w) in NCH384:
                    qk_chunk(2, c0, w, ps1)      # k chunk 0 (heads 0-3)
                kpad_dma(range(2))
                for (c0, w) in NCH384[:3]:
                    qk_chunk(0, c0, w, ps1)      # q chunk 0
                kpad_dma(range(2, 4))
                vt_chunk(0, ps1)
                vt_chunk(1, ps1)

            with tc.tile_pool(name="drp", bufs=1, space="DRAM") as drp:
                rld = drp.tile([NHEADS, NH], f32, tag="rld", name="rld")

            def emit_norm(oc, c0, w):
                # softmax denominators -> reciprocals -> broadcast -> normalize,
                # add bias + positional conv. All DVE/DMA: overlaps attention.
                nc.vector.reciprocal_approx_accurate(rl_g[oc][:, c0:c0 + w],
                                                     l_g[oc][:, c0:c0 + w],
                                                     rscr[:, c0:c0 + w])
                for g in range(4):
                    h = 4 * oc + g
                    nc.sync.dma_start(out=rld[h:h + 1, c0:c0 + w],
                                      in_=rl_g[oc][g:g + 1, c0:c0 + w])
                    nc.sync.dma_start(
                        out=rlb_sb[oc][32 * g:32 * (g + 1), c0:c0 + w],
                        in_=rld[h:h + 1, c0:c0 + w].partition_broadcast(32),
                    )
                pef = pe[oc][:, :, :].rearrange("p a b -> p (a b)")
                nc.vector.tensor_tensor(
                    proj_in[oc][:, c0:c0 + w], outU[oc][:, c0:c0 + w],
                    rlb_sb[oc][:, c0:c0 + w], OP.mult,
                )
                nc.vector.scalar_tensor_tensor(
                    proj_in[oc][:, c0:c0 + w], proj_in[oc][:, c0:c0 + w],
                    bvpe[oc][:, :], pef[:, c0:c0 + w], OP.add, OP.add,
                )

            y_sb = [pp.tile([128, NH], f32, tag=f"y{c}", name=f"y{c}") for c in range(2)]

            # ================= stage 2: attention + fillers ============
            with (
                tc.tile_pool(name="ep", bufs=3) as ep,
                tc.tile_pool(name="stp", bufs=2, space="PSUM") as stp,
                tc.tile_pool(name="ava", bufs=1, space="PSUM") as ava,
                tc.tile_pool(name="ps3", bufs=1, space="PSUM") as ps3,
            ):
                def emit_proj(c0, w):
                    for oc in range(2):
                        pt = ps3.tile([128, 384], f32, tag="pps", name="pps")
                        for c in range(2):
                            nc.tensor.matmul(
                                pt[:, :w],
                                wprojT[c][:, 128 * oc:128 * (oc + 1)],
                                proj_in[c][:, c0:c0 + w],
                                start=(c == 0), stop=(c == 1),
                            )
                        nc.vector.tensor_scalar(y_sb[oc][:, c0:c0 + w], pt[:, :w],
                                                bproj[oc][:, :], None, OP.add)
                        nc.sync.dma_start(out=out_d[128 * oc:128 * (oc + 1), c0:c0 + w],
                                          in_=y_sb[oc][:, c0:c0 + w])

                FILL = {}
                FILL[(0, 0)] = [(lambda j=j: vt_chunk(j, ps3)) for j in range(2, NJ)]
                FILL[(0, 1)] = [(lambda c0=c0, w=w: qk_chunk(3, c0, w, ps3))
                                for (c0, w) in NCH384]
                FILL[(0, 2)] = ([(lambda c0=c0, w=w: qk_chunk(1, c0, w, ps3))
                                 for (c0, w) in NCH384[:3]]
                                + [lambda: kpad_dma(range(4, 8))])
                FILL[(1, 0)] = ([(lambda c0=c0, w=w: v4_chunk(0, c0, w, ps3))
                                 for (c0, w) in [(0, 384), (384, 384), (768, 384), (1152, 48)]]
                                + [lambda: v4_halo(0, ps3), lambda: pe_taps(0)])
                FILL[(1, 1)] = ([(lambda c0=c0, w=w: v4_chunk(1, c0, w, ps3))
                                 for (c0, w) in [(0, 384), (384, 384), (768, 384), (1152, 48)]]
                                + [lambda: v4_halo(1, ps3), lambda: pe_taps(1)])

                for grp in range(4):
                    heads = [2 * grp, 2 * grp + 1]
                    oc = heads[0] // 4
                    units = []
                    for j in range(NJ):
                        for h in heads:
                            units.append((h, j))
                    triples = [units[3 * t:3 * t + 3] for t in range(len(units) // 3)]
                    for ici, (i0, icw) in enumerate(ISUBS):
                        fillers = FILL.get((grp, ici), [])
                        fi = 0
                        avl = ava.tile([97, 384], f32, tag="avla", name="avla")
                        ets = {}

                        def emit_st(t):
                            st = stp.tile([128, 3, 512], f32, tag="st", name="st")
                            et = ep.tile([128, 3, 512], bf16, tag="E", name="E")
                            use_dve = (t % 3 == 2)
                            for s, (h, j) in enumerate(triples[t]):
                                if s == 0:
                                    # full-K matmul (zero-padded k) keeps the
                                    # PE activity monitor warm
                                    nc.tensor.matmul(
                                        st[:, s, :icw],
                                        k_pad[h][:, 128 * j:128 * (j + 1)],
                                        q_sb[oc][:, i0:i0 + icw],
                                        start=True, stop=True,
                                    )
                                else:
                                    r = 32 * (h % 4)
                                    nc.tensor.matmul(
                                        st[:, s, :icw],
                                        k_sb[oc][r:r + 32, 128 * j:128 * (j + 1)],
                                        q_sb[oc][r:r + 32, i0:i0 + icw],
                                        start=True, stop=True,
                                        tile_position=(r, 0),
                                    )
                            if use_dve:
                                etu = et[:, :, :].bitcast(mybir.dt.uint16)
                                nc.vector.tensor_scalar(
                                    etu[:, :, :icw], st[:, :, :icw],
                                    128.0 * LOG2E * SCALE, EXPB16,
                                    OP.mult, OP.add)
                            else:
                                nc.scalar.activation(et[:, :, :icw], st[:, :, :icw],
                                                     AF.Exp, scale=SCALE)
                            ets[t] = et

                        def emit_av(t):
                            et = ets.pop(t)
                            for s, (h, j) in enumerate(triples[t]):
                                cp = 64 * (h % 2)
                                nc.tensor.matmul(
                                    avl[cp:cp + 33, :icw],
                                    vT[:, j, h, 0:33], et[:, s, :icw],
                                    start=(j == 0), stop=(j == NJ - 1),
                                    tile_position=(0, cp),
                                )

                        for t in range(len(triples)):
                            emit_st(t)
                            for _ in range(2):
                                if fi < len(fillers):
                                    fillers[fi]()
                                    fi += 1
                            if t >= 1:
                                emit_av(t - 1)
                        while fi < len(fillers):
                            fillers[fi]()
                            fi += 1
                        emit_av(len(triples) - 1)

                        for h in heads:
                            g = h % 4
                            cp = 64 * (h % 2)
                            rr = 32 * g
                            nc.vector.tensor_copy(outU[oc][rr:rr + 32, i0:i0 + icw],
                                                  avl[cp:cp + 32, :icw])
                            lt = ep.tile([1, 384], f32, tag="ltmp", name="ltmp")
                            nc.vector.tensor_copy(lt[:, :icw], avl[cp + 32:cp + 33, :icw])
                            nc.sync.dma_start(out=l_g[oc][g:g + 1, i0:i0 + icw],
                                              in_=lt[:, :icw])
                        if grp % 2 == 1:
                            emit_norm(oc, i0, icw)
                        if grp == 3:
                            emit_proj(i0, icw)
            xpool.__exit__(None, None, None)

    nc.finalize()
    return nc


def _prep_inputs(x, w_qk, b_qk, w_v, b_v, w_pe, b_pe, w_proj, b_proj):
    f = np.float32
    base = np.zeros((C, WPACK), dtype=f)
    # reference reshapes qk conv output to (h, 2d): channels 64h..64h+32 are
    # q_h, 64h+32..64h+64 are k_h. Repack host-side to [q by head | k by head].
    wqk2 = w_qk[:, :, 0, 0].reshape(NHEADS, 2 * D, C)
    bqk2 = b_qk.reshape(NHEADS, 2 * D)
    wq = wqk2[:, :D].reshape(C, C)
    wk = wqk2[:, D:].reshape(C, C)
    base[:, 0:256] = wq.T
    base[:, 256:512] = wk.T
    base[:, 512:768] = w_v[:, :, 0, 0].T
    base[:, 768:1024] = w_proj[:, :, 0, 0].T
    base[:, 1024:1033] = w_pe[:, 0].reshape(C, 9)
    base[:, 1033] = bqk2[:, :D].reshape(C)
    base[:, 1034] = bqk2[:, D:].reshape(C)
    base[:, 1035] = b_v
    base[:, 1036] = b_v + b_pe
    base[:, 1037] = b_proj
    wpackh = np.ascontiguousarray(base[:, 0:1024].astype(bfloat16))
    kzero = np.zeros((C, N), dtype=bfloat16)

    in_maps = []
    for core in range(8):
        b, half = core // 2, core % 2
        y0 = 24 * half
        xb = x[b].reshape(C, HW, HW).astype(f)
        xr = np.concatenate([xb[:, y0:, :], xb[:, :y0, :]], axis=1)
        halo_top = 1.0 if half == 1 else 0.0
        halo_bot = 1.0 if half == 0 else 0.0
        wpack = base.copy()
        wpack[:, 1038] = halo_top * b_v
        wpack[:, 1039] = halo_top
        wpack[:, 1040] = halo_bot
        in_maps.append({
            "x": np.ascontiguousarray(xr.reshape(C, N).astype(bfloat16)),
            "wpack": wpack, "wpackh": wpackh, "kzero": kzero,
        })
    return in_maps


def kernel(**inputs):
    from concourse.bass_utils import run_bass_kernel_spmd

    if "nc" not in _CACHE:
        _CACHE["nc"] = _build_bass()
    nc = _CACHE["nc"]

    in_maps = _prep_inputs(**inputs)
    res = run_bass_kernel_spmd(nc, in_maps, core_ids=list(range(8)))
    y = np.empty((4, C, HW, HW), dtype=np.float32)
    for core in range(8):
        b, half = core // 2, core % 2
        y0 = 24 * half
        y[b][:, y0:y0 + 24, :] = res.results[core]["out"].reshape(C, 24, HW)
    return y



# revision 30
# speedup vs baseline: 1.3649x; 1.1437x over previous
"""Trainium2 Bass kernel for nn_AttentionV2 (dense transformer attention block).

Reference computation (B=4, C=256, H=W=48, heads=8, d=32, N=2304):
  qk   = conv1x1(x, w_qk) -> q,k per head [d, N]
  v4   = conv1x1(x, w_v)
  pe   = dwconv3x3(v4, w_pe)            (depthwise, SAME)
  S    = q^T k * d^-0.5 ; P = softmax_j(S)
  out  = v @ P^T  (per head)
  y    = conv1x1(out + pe, w_proj)

Sharding: 8 cores = 4 batches x 2 spatial halves (rows 0-23 / 24-47).
Each core computes full k,v for its batch (attention rows are independent
given full k/v) -> zero collectives. The per-core x is ROLLED by y0 rows so
the SPMD program always works on "rows 0..23"; softmax/attention are
permutation-invariant in j. Halo rows for the depthwise conv are gated by
per-core 0/1 flag inputs.

Attention is computed in "ST" layout (S^T = k^T q, j on partitions):
 - PE matmul k[:,jc]^T q -> ST psum [128 j, 1152 i]
 - ACT exp(SCALE*ST) psum -> SBUF E (bf16), no max-subtraction (|S*scale|<~5)
 - PE AV matmul with lhsT = [v^T | ones] accumulated over j-chunks gives both
   the unnormalized output (rows 0..32) and the softmax denominator (row 32).
"""

import os
import numpy as np
from ml_dtypes import bfloat16

C = 256
HW = 48
N = HW * HW          # 2304
NH = N // 2          # 1152 per-core i-pixels
NHEADS = 8
D = 32
SCALE = D ** -0.5
NJ = N // 128        # 18 j-chunks
ISUBS = [(0, 384), (384, 384), (768, 384)]
NCH384 = [(k * 384, 384) for k in range(6)]   # full-image conv col chunks
WPACK = 1041

_CACHE = {}


def _build_bass():
    import concourse.bass as bass
    import concourse.bacc as bacc
    import concourse.mybir as mybir
    from concourse import tile

    f32 = mybir.dt.float32
    bf16 = mybir.dt.bfloat16
    AF = mybir.ActivationFunctionType
    OP = mybir.AluOpType

    nc = bacc.Bacc()

    x_d = nc.dram_tensor("x", [C, N], bf16, kind="ExternalInput")
    # all weights/biases packed into one tensor: [wqkT|wvT|wprojT|wpe|biases|halo]
    wpack_d = nc.dram_tensor("wpack", [C, WPACK], f32, kind="ExternalInput")
    # bf16 copy of the matmul weights [wqkT|wvT|wprojT]
    wpackh_d = nc.dram_tensor("wpackh", [C, 1024], bf16, kind="ExternalInput")
    kzero_d = nc.dram_tensor("kzero", [C, N], bf16, kind="ExternalInput")
    out_d = nc.dram_tensor("out", [C, NH], f32, kind="ExternalOutput")

    with tile.TileContext(nc) as tc:
        with (
            tc.tile_pool(name="wts", bufs=1) as wp,
            tc.tile_pool(name="per", bufs=1) as pp,
        ):
            # ---- persistent weight/bias tiles (single packed DMA per chunk)
            wsb = [wp.tile([128, WPACK], f32, tag=f"wsb{c}", name=f"wsb{c}") for c in range(2)]
            wph = [wp.tile([128, 1024], bf16, tag=f"wph{c}", name=f"wph{c}") for c in range(2)]
            for c in range(2):
                nc.sync.dma_start(out=wsb[c][:, :], in_=wpack_d[128 * c:128 * (c + 1), :])
                nc.sync.dma_start(out=wph[c][:, :], in_=wpackh_d[128 * c:128 * (c + 1), :])
            wqkT = [wph[c][:, 0:512] for c in range(2)]
            wvT = [wph[c][:, 512:768] for c in range(2)]
            wprojT = [wph[c][:, 768:1024] for c in range(2)]
            wpe = [wsb[c][:, 1024:1033] for c in range(2)]
            bq = [wsb[c][:, 1033:1034] for c in range(2)]
            bk = [wsb[c][:, 1034:1035] for c in range(2)]
            bv = [wsb[c][:, 1035:1036] for c in range(2)]
            bvpe = [wsb[c][:, 1036:1037] for c in range(2)]
            bproj = [wsb[c][:, 1037:1038] for c in range(2)]
            bvht = [wsb[c][:, 1038:1039] for c in range(2)]
            halo = [wsb[c][:, 1039:1041] for c in range(2)]

            # ---- persistent activations
            q_sb = [pp.tile([128, NH], bf16, tag=f"q{c}", name=f"q{c}") for c in range(2)]
            k_sb = [pp.tile([128, N], bf16, tag=f"k{c}", name=f"k{c}") for c in range(2)]
            vT = pp.tile([128, NJ, NHEADS, 33], bf16, tag="vT", name="vT")
            v4 = [pp.tile([128, 26, HW], f32, tag=f"v4{c}", name=f"v4{c}") for c in range(2)]
            htop = [pp.tile([128, 1, HW], f32, tag=f"htop{c}", name=f"htop{c}") for c in range(2)]
            hbot = [pp.tile([128, 1, HW], f32, tag=f"hbot{c}", name=f"hbot{c}") for c in range(2)]
            pe = [pp.tile([128, 24, HW], f32, tag=f"pe{c}", name=f"pe{c}") for c in range(2)]
            outU = [pp.tile([128, NH], f32, tag=f"outU{c}", name=f"outU{c}") for c in range(2)]
            l_g = [pp.tile([4, NH], f32, tag=f"l{g}", name=f"l{g}") for g in range(2)]
            rl_g = [pp.tile([4, NH], f32, tag=f"rl{g}", name=f"rl{g}") for g in range(2)]
            rscr = pp.tile([4, NH], f32, tag="rscr", name="rscr")
            rl8 = [pp.tile([1, NH], f32, tag=f"rl8_{h}", name=f"rl8_{h}") for h in range(NHEADS)]
            rlb_sb = [pp.tile([128, NH], f32, tag=f"rlb{g}", name=f"rlb{g}") for g in range(2)]
            proj_in = [pp.tile([128, NH], bf16, tag=f"pin{c}", name=f"pin{c}") for c in range(2)]

            nc.vector.memset(vT[:, :, :, 32:33], 1.0)
            k_pad = [pp.tile([128, N], bf16, tag=f"kp{h}", name=f"kp{h}")
                     for h in range(NHEADS)]
            for h in range(2):
                nc.sync.dma_start(out=k_pad[h][:, :], in_=kzero_d[0:128, :])

            # ================= stage 1: minimal prefix =================
            # Only what the first ST/exp needs runs before attention: x, the
            # k/q convs for heads 0/1 and the first two vT chunks. Everything
            # else becomes "filler" closures interleaved into the attention
            # loop so the PE never idles and the first exp starts early.
            xpool = tc.tile_pool(name="xp", bufs=1)
            xp = xpool.__enter__()
            x_sb = [xp.tile([128, N], bf16, tag=f"x{c}", name=f"x{c}") for c in range(2)]
            for s in range(0, N, 768):
                for c in range(2):
                    nc.sync.dma_start(out=x_sb[c][:, s:s + 768],
                                      in_=x_d[128 * c:128 * (c + 1), s:s + 768])

            def qk_chunk(t, c0, w, pool):
                is_q = t < 2
                oc = t % 2
                dst = q_sb[oc] if is_q else k_sb[oc]
                bias = bq[oc] if is_q else bk[oc]
                pt = pool.tile([128, 384], f32, tag="pps", name="pps")
                for c in range(2):
                    nc.tensor.matmul(
                        pt[:, :w],
                        wqkT[c][:, 128 * t:128 * (t + 1)],
                        x_sb[c][:, c0:c0 + w],
                        start=(c == 0), stop=(c == 1),
                    )
                nc.vector.tensor_scalar(dst[:, c0:c0 + w], pt[:, :w], bias[:, :], None, OP.add)

            def vt_chunk(j, pool):
                pt = pool.tile([128, 384], f32, tag="pps", name="pps")
                for c in range(2):
                    nc.tensor.matmul(
                        pt[:, :256],
                        x_sb[c][:, 128 * j:128 * (j + 1)],
                        wvT[c][:, :],
                        start=(c == 0), stop=(c == 1),
                    )
                nc.vector.tensor_copy(
                    vT[:, j, :, 0:32],
                    pt[:, :256].rearrange("p (h d) -> p h d", d=32),
                )

            def v4_chunk(oc, c0, w, pool):
                pt = pool.tile([128, 384], f32, tag="pps", name="pps")
                for c in range(2):
                    nc.tensor.matmul(
                        pt[:, :w],
                        wvT[c][:, 128 * oc:128 * (oc + 1)],
                        x_sb[c][:, c0:c0 + w],
                        start=(c == 0), stop=(c == 1),
                    )
                v4f = v4[oc][:, :, :].rearrange("p a b -> p (a b)")
                nc.vector.tensor_scalar(v4f[:, c0:c0 + w], pt[:, :w], bv[oc][:, :], None, OP.add)

            def v4_halo(oc, pool):
                pt = pool.tile([128, 384], f32, tag="pps", name="pps")
                for c in range(2):
                    nc.tensor.matmul(
                        pt[:, :48],
                        wvT[c][:, 128 * oc:128 * (oc + 1)],
                        x_sb[c][:, 47 * 48:48 * 48],
                        start=(c == 0), stop=(c == 1),
                    )
                nc.scalar.activation(htop[oc][:, 0, :], pt[:, :48], AF.Identity,
                                     bias=bvht[oc][:, :], scale=halo[oc][:, 0:1])
                nc.scalar.activation(hbot[oc][:, 0, :], v4[oc][:, 24, :], AF.Copy,
                                     scale=halo[oc][:, 1:2])

            def pe_taps(oc):
                w9 = wpe[oc]
                acc = pe[oc]
                src = v4[oc]
                nc.vector.tensor_scalar(acc[:, :, :], src[:, 0:24, :], w9[:, 4:5], None, OP.mult)
                taps = [
                    (-1, -1, (1, 24), (0, 23), (1, 48), (0, 47)),
                    (-1, 0, (1, 24), (0, 23), (0, 48), (0, 48)),
                    (-1, 1, (1, 24), (0, 23), (0, 47), (1, 48)),
                    (0, -1, (0, 24), (0, 24), (1, 48), (0, 47)),
                    (0, 1, (0, 24), (0, 24), (0, 47), (1, 48)),
                    (1, -1, (0, 23), (1, 24), (1, 48), (0, 47)),
                    (1, 0, (0, 23), (1, 24), (0, 48), (0, 48)),
                    (1, 1, (0, 23), (1, 24), (0, 47), (1, 48)),
                ]
                for (dy, dx, oy, iy, ox, ix) in taps:
                    wap = w9[:, 3 * (dy + 1) + (dx + 1):3 * (dy + 1) + (dx + 1) + 1]
                    nc.vector.scalar_tensor_tensor(
                        acc[:, oy[0]:oy[1], ox[0]:ox[1]],
                        src[:, iy[0]:iy[1], ix[0]:ix[1]],
                        wap,
                        acc[:, oy[0]:oy[1], ox[0]:ox[1]],
                        OP.mult, OP.add,
                    )
                for (dx, ox, ix) in [(-1, (1, 48), (0, 47)), (0, (0, 48), (0, 48)), (1, (0, 47), (1, 48))]:
                    wap = w9[:, (dx + 1):(dx + 2)]
                    nc.vector.scalar_tensor_tensor(
                        acc[:, 0:1, ox[0]:ox[1]], htop[oc][:, :, ix[0]:ix[1]],
                        wap, acc[:, 0:1, ox[0]:ox[1]], OP.mult, OP.add,
                    )
                    wap = w9[:, 6 + (dx + 1):6 + (dx + 2)]
                    nc.vector.scalar_tensor_tensor(
                        acc[:, 23:24, ox[0]:ox[1]], hbot[oc][:, :, ix[0]:ix[1]],
                        wap, acc[:, 23:24, ox[0]:ox[1]], OP.mult, OP.add,
                    )

            def kpad_dma(heads):
                for h in heads:
                    if h >= 2:
                        nc.sync.dma_start(out=k_pad[h][:, :], in_=kzero_d[0:128, :])
                for h in heads:
                    r = 32 * (h % 4)
                    nc.sync.dma_start(out=k_pad[h][r:r + 32, :],
                                      in_=k_sb[h // 4][r:r + 32, :])

            with tc.tile_pool(name="ps1", bufs=2, space="PSUM") as ps1:
                for (c0, w) in NCH384:
                    qk_chunk(2, c0, w, ps1)      # k chunk 0 (heads 0-3)
                kpad_dma(range(2))
                for (c0, w) in NCH384[:3]:
                    qk_chunk(0, c0, w, ps1)      # q chunk 0
                kpad_dma(range(2, 4))
                vt_chunk(0, ps1)
                vt_chunk(1, ps1)

            with tc.tile_pool(name="drp", bufs=1, space="DRAM") as drp:
                rld = drp.tile([NHEADS, NH], f32, tag="rld", name="rld")

            def emit_norm(oc, c0, w):
                # softmax denominators -> reciprocals -> broadcast -> normalize,
                # add bias + positional conv. All DVE/DMA: overlaps attention.
                nc.vector.reciprocal_approx_accurate(rl_g[oc][:, c0:c0 + w],
                                                     l_g[oc][:, c0:c0 + w],
                                                     rscr[:, c0:c0 + w])
                for g in range(4):
                    h = 4 * oc + g
                    nc.sync.dma_start(out=rld[h:h + 1, c0:c0 + w],
                                      in_=rl_g[oc][g:g + 1, c0:c0 + w])
                    nc.sync.dma_start(
                        out=rlb_sb[oc][32 * g:32 * (g + 1), c0:c0 + w],
                        in_=rld[h:h + 1, c0:c0 + w].partition_broadcast(32),
                    )
                pef = pe[oc][:, :, :].rearrange("p a b -> p (a b)")
                nc.vector.tensor_tensor(
                    proj_in[oc][:, c0:c0 + w], outU[oc][:, c0:c0 + w],
                    rlb_sb[oc][:, c0:c0 + w], OP.mult,
                )
                nc.vector.scalar_tensor_tensor(
                    proj_in[oc][:, c0:c0 + w], proj_in[oc][:, c0:c0 + w],
                    bvpe[oc][:, :], pef[:, c0:c0 + w], OP.add, OP.add,
                )

            y_sb = [pp.tile([128, NH], f32, tag=f"y{c}", name=f"y{c}") for c in range(2)]

            # ================= stage 2: attention + fillers ============
            with (
                tc.tile_pool(name="ep", bufs=3) as ep,
                tc.tile_pool(name="stp", bufs=2, space="PSUM") as stp,
                tc.tile_pool(name="ava", bufs=1, space="PSUM") as ava,
                tc.tile_pool(name="ps3", bufs=1, space="PSUM") as ps3,
            ):
                def emit_proj(c0, w):
                    for oc in range(2):
                        pt = ps3.tile([128, 384], f32, tag="pps", name="pps")
                        for c in range(2):
                            nc.tensor.matmul(
                                pt[:, :w],
                                wprojT[c][:, 128 * oc:128 * (oc + 1)],
                                proj_in[c][:, c0:c0 + w],
                                start=(c == 0), stop=(c == 1),
                            )
                        nc.vector.tensor_scalar(y_sb[oc][:, c0:c0 + w], pt[:, :w],
                                                bproj[oc][:, :], None, OP.add)
                        nc.sync.dma_start(out=out_d[128 * oc:128 * (oc + 1), c0:c0 + w],
                                          in_=y_sb[oc][:, c0:c0 + w])

                FILL = {}
                FILL[(0, 0)] = [(lambda j=j: vt_chunk(j, ps3)) for j in range(2, NJ)]
                FILL[(0, 1)] = [(lambda c0=c0, w=w: qk_chunk(3, c0, w, ps3))
                                for (c0, w) in NCH384]
                FILL[(0, 2)] = ([(lambda c0=c0, w=w: qk_chunk(1, c0, w, ps3))
                                 for (c0, w) in NCH384[:3]]
                                + [lambda: kpad_dma(range(4, 8))])
                FILL[(1, 0)] = ([(lambda c0=c0, w=w: v4_chunk(0, c0, w, ps3))
                                 for (c0, w) in [(0, 384), (384, 384), (768, 384), (1152, 48)]]
                                + [lambda: v4_halo(0, ps3), lambda: pe_taps(0)])
                FILL[(1, 1)] = ([(lambda c0=c0, w=w: v4_chunk(1, c0, w, ps3))
                                 for (c0, w) in [(0, 384), (384, 384), (768, 384), (1152, 48)]]
                                + [lambda: v4_halo(1, ps3), lambda: pe_taps(1)])

                for grp in range(4):
                    heads = [2 * grp, 2 * grp + 1]
                    oc = heads[0] // 4
                    units = []
                    for j in range(NJ):
                        for h in heads:
                            units.append((h, j))
                    triples = [units[3 * t:3 * t + 3] for t in range(len(units) // 3)]
                    for ici, (i0, icw) in enumerate(ISUBS):
                        fillers = FILL.get((grp, ici), [])
                        fi = 0
                        avl = ava.tile([97, 384], f32, tag="avla", name="avla")
                        ets = {}

                        def emit_st(t):
                            st = stp.tile([128, 3, 512], f32, tag="st", name="st")
                            et = ep.tile([128, 3, 512], bf16, tag="E", name="E")
                            for s, (h, j) in enumerate(triples[t]):
                                if s == 0:
                                    # full-K matmul (zero-padded k) keeps the
                                    # PE activity monitor warm
                                    nc.tensor.matmul(
                                        st[:, s, :icw],
                                        k_pad[h][:, 128 * j:128 * (j + 1)],
                                        q_sb[oc][:, i0:i0 + icw],
                                        start=True, stop=True,
                                    )
                                else:
                                    r = 32 * (h % 4)
                                    nc.tensor.matmul(
                                        st[:, s, :icw],
                                        k_sb[oc][r:r + 32, 128 * j:128 * (j + 1)],
                                        q_sb[oc][r:r + 32, i0:i0 + icw],
                                        start=True, stop=True,
                                        tile_position=(r, 0),
                                    )
                            nc.scalar.activation(et[:, :, :icw], st[:, :, :icw],
                                                 AF.Exp, scale=SCALE)
                            ets[t] = et

                        def emit_av(t):
                            et = ets.pop(t)
                            for s, (h, j) in enumerate(triples[t]):
                                cp = 64 * (h % 2)
                                nc.tensor.matmul(
                                    avl[cp:cp + 33, :icw],
                                    vT[:, j, h, 0:33], et[:, s, :icw],
                                    start=(j == 0), stop=(j == NJ - 1),
                                    tile_position=(0, cp),
                                )

                        for t in range(len(triples)):
                            emit_st(t)
                            for _ in range(2):
                                if fi < len(fillers):
                                    fillers[fi]()
                                    fi += 1
                            if t >= 1:
                                emit_av(t - 1)
                        while fi < len(fillers):
                            fillers[fi]()
                            fi += 1
                        emit_av(len(triples) - 1)

                        for h in heads:
                            g = h % 4
                            cp = 64 * (h % 2)
                            rr = 32 * g
                            nc.vector.tensor_copy(outU[oc][rr:rr + 32, i0:i0 + icw],
                                                  avl[cp:cp + 32, :icw])
                            lt = ep.tile([1, 384], f32, tag="ltmp", name="ltmp")
                            nc.vector.tensor_copy(lt[:, :icw], avl[cp + 32:cp + 33, :icw])
                            nc.sync.dma_start(out=l_g[oc][g:g + 1, i0:i0 + icw],
                                              in_=lt[:, :icw])
                        if grp % 2 == 1:
                            emit_norm(oc, i0, icw)
                        if grp == 3:
                            emit_proj(i0, icw)
            xpool.__exit__(None, None, None)

    nc.finalize()
    return nc


def _prep_inputs(x, w_qk, b_qk, w_v, b_v, w_pe, b_pe, w_proj, b_proj):
    f = np.float32
    base = np.zeros((C, WPACK), dtype=f)
    # reference reshapes qk conv output to (h, 2d): channels 64h..64h+32 are
    # q_h, 64h+32..64h+64 are k_h. Repack host-side to [q by head | k by head].
    wqk2 = w_qk[:, :, 0, 0].reshape(NHEADS, 2 * D, C)
    bqk2 = b_qk.reshape(NHEADS, 2 * D)
    wq = wqk2[:, :D].reshape(C, C)
    wk = wqk2[:, D:].reshape(C, C)
    base[:, 0:256] = wq.T
    base[:, 256:512] = wk.T
    base[:, 512:768] = w_v[:, :, 0, 0].T
    base[:, 768:1024] = w_proj[:, :, 0, 0].T
    base[:, 1024:1033] = w_pe[:, 0].reshape(C, 9)
    base[:, 1033] = bqk2[:, :D].reshape(C)
    base[:, 1034] = bqk2[:, D:].reshape(C)
    base[:, 1035] = b_v
    base[:, 1036] = b_v + b_pe
    base[:, 1037] = b_proj
    wpackh = np.ascontiguousarray(base[:, 0:1024].astype(bfloat16))
    kzero = np.zeros((C, N), dtype=bfloat16)

    in_maps = []
    for core in range(8):
        b, half = core // 2, core % 2
        y0 = 24 * half
        xb = x[b].reshape(C, HW, HW).astype(f)
        xr = np.concatenate([xb[:, y0:, :], xb[:, :y0, :]], axis=1)
        halo_top = 1.0 if half == 1 else 0.0
        halo_bot = 1.0 if half == 0 else 0.0
        wpack = base.copy()
        wpack[:, 1038] = halo_top * b_v
        wpack[:, 1039] = halo_top
        wpack[:, 1040] = halo_bot
        in_maps.append({
            "x": np.ascontiguousarray(xr.reshape(C, N).astype(bfloat16)),
            "wpack": wpack, "wpackh": wpackh, "kzero": kzero,
        })
    return in_maps


def kernel(**inputs):
    from concourse.bass_utils import run_bass_kernel_spmd

    if "nc" not in _CACHE:
        _CACHE["nc"] = _build_bass()
    nc = _CACHE["nc"]

    in_maps = _prep_inputs(**inputs)
    res = run_bass_kernel_spmd(nc, in_maps, core_ids=list(range(8)))
    y = np.empty((4, C, HW, HW), dtype=np.float32)
    for core in range(8):
        b, half = core // 2, core % 2
        y0 = 24 * half
        y[b][:, y0:y0 + 24, :] = res.results[core]["out"].reshape(C, 24, HW)
    return y

